# revision 8
# baseline (speedup 1.0000x reference)
"""Bahdanau attention TRN2 kernel (v2: fp16 keys + DMA-transpose).

Full inputs -> full outputs. Data-parallel over batch B=32 across 8 cores
(4 batches/core). Weights replicated (tiny, pre-transposed/cast on host).

Per core, per 512-token block (4 blocks/batch):
  DMA  keys tiles [128,1024] fp16 (natural layout, for context matmul)
  DMA  keysT [128 d, 512 tok] fp16 via x-bar DMA transpose (for k_proj)
  PE   mm1: k_projT[h, tok] = sum_d WkT[d,h].T @ keysT[d,tok]   (fp16 -> fp32 psum)
  ACT  tanh(k_proj + q_proj + Wk_b) with per-partition bias column
  PE   mm2: scores[1, tok] = sum_h VaT[h,1].T @ tanh[h,tok]      (fp32r)
  ACT  exp (no max subtract; scores bounded ~|4| for N(0,1) inputs)
       with fused accum_out partial denominator
  PE   transpose e-row chunks -> e columns [tok,1] (tiny)
  PE   mm3: ctx[1, d] += e[tok,1].T @ keys[tok,d]                (fp16)
Softmax bookkeeping lives entirely on partition 0 (no cross-partition ops).
weights = e * (1/denom); ctx scaled by 1/denom per batch.
"""

import numpy as np

B, S, H = 32, 2048, 512
D2 = 2 * H          # 1024
NCORES = 8
BPC = B // NCORES   # 4 batches per core
NBLK = 4            # 512-token blocks per batch
TPB = 4             # 128-token tiles per block

_CACHE = {}


def _build():
    import concourse.bacc as bacc
    import concourse.tile as tile
    from concourse import mybir

    f16 = mybir.dt.float16
    f32 = mybir.dt.float32
    f32r = mybir.dt.float32r
    AF = mybir.ActivationFunctionType

    nc = bacc.Bacc(None, target_bir_lowering=False)

    keys_d = nc.dram_tensor("keys16", [BPC, S, D2], f16, kind="ExternalInput")
    wkT_d = nc.dram_tensor("wkT", [8, 128, H], f16, kind="ExternalInput")
    qkbT_d = nc.dram_tensor("qkbT", [4, 128, BPC], f32, kind="ExternalInput")
    vaT_d = nc.dram_tensor("vaT", [4, 128], f32r, kind="ExternalInput")
    ident_d = nc.dram_tensor("ident", [1, 1], f32, kind="ExternalInput")
    ctx_d = nc.dram_tensor("ctx", [1, BPC, D2], f32, kind="ExternalOutput")
    wout_d = nc.dram_tensor("wout", [1, BPC, S], f32, kind="ExternalOutput")
    er_d = nc.dram_tensor("er_dbg", [1, BPC, S], f32, kind="ExternalOutput")
    dp_d = nc.dram_tensor("dp_dbg", [1, BPC, NBLK], f32, kind="ExternalOutput")

    keys_r = keys_d.rearrange("b (t p) d -> b t p d", p=128)  # t = 16 token tiles

    with tile.TileContext(nc) as tc:
        with (
            tc.tile_pool(name="const", bufs=1) as constp,
            tc.tile_pool(name="keys", bufs=8) as keysp,
            tc.tile_pool(name="keysT", bufs=2) as keysTp,
            tc.tile_pool(name="tt", bufs=2) as ttp,
            tc.tile_pool(name="small", bufs=1) as smallp,
            tc.tile_pool(name="eTs", bufs=2) as eTsp,
            tc.tile_pool(name="pkp", bufs=3, space="PSUM") as pkpp,
            tc.tile_pool(name="psc", bufs=2, space="PSUM") as pscp,
            tc.tile_pool(name="peT", bufs=1, space="PSUM") as peTp,
            tc.tile_pool(name="pctx", bufs=1, space="PSUM") as pctxp,
        ):
            # ---- constants ----
            wkT = constp.tile([128, 8, H], f16)       # [d_in_chunk, di, h]
            qkbT = constp.tile([128, 4, BPC], f32)    # [h_in_chunk, hi, b]
            vaT = constp.tile([128, 4], f32r)         # [h_in_chunk, hi]
            ident = constp.tile([1, 1], f32)
            nc.gpsimd.dma_start(out=wkT, in_=wkT_d.rearrange("k p h -> p k h"))
            nc.gpsimd.dma_start(out=qkbT, in_=qkbT_d.rearrange("k p b -> p k b"))
            nc.gpsimd.dma_start(out=vaT, in_=vaT_d.rearrange("k p -> p k"))
            nc.gpsimd.dma_start(out=ident, in_=ident_d[:, :])

            # softmax bookkeeping, all on partition 0
            e_rows = smallp.tile([1, BPC, S], f32)
            w_rows = smallp.tile([1, BPC, S], f32)
            dparts = smallp.tile([1, BPC, NBLK], f32)
            denom = smallp.tile([1, BPC], f32)
            rinv = smallp.tile([1, BPC], f32)
            ctx_sb = smallp.tile([1, BPC, D2], f32)

            for b in range(BPC):
                ctx_ps = pctxp.tile([1, D2], f32)
                for blk in range(NBLK):
                    tok0 = 512 * blk
                    # ---- load 4 natural keys tiles + 8 transposed chunks ----
                    ktiles = []
                    for tt in range(TPB):
                        kt = keysp.tile([128, D2], f16, tag="ktile")
                        nc.gpsimd.dma_start(out=kt, in_=keys_r[b, blk * TPB + tt])
                        ktiles.append(kt)
                    keysT = keysTp.tile([128, 8, 512], f16)
                    for di in range(8):
                        nc.sync.dma_start_transpose(
                            out=keysT[:, di, :],
                            in_=keys_d[b, tok0:tok0 + 512,
                                       128 * di:128 * (di + 1)],
                        )

                    # ---- mm1 + tanh + mm2 ----
                    score_ps = pscp.tile([1, 512], f32)
                    for hi in range(4):
                        kp = pkpp.tile([128, 512], f32)
                        for di in range(8):
                            nc.tensor.matmul(
                                kp[:, :],
                                wkT[:, di, 128 * hi:128 * (hi + 1)],
                                keysT[:, di, :],
                                start=(di == 0), stop=(di == 7),
                            )
                        th = ttp.tile([128, 512], f32r)
                        nc.scalar.activation(
                            out=th, in_=kp[:, :], func=AF.Tanh,
                            bias=qkbT[:, hi, b:b + 1],
                        )
                        nc.tensor.matmul(
                            score_ps[:, :],
                            vaT[:, hi:hi + 1],
                            th[:, :],
                            start=(hi == 0), stop=(hi == 3),
                        )

                    # ---- exp + partial denom (fused accumulate) ----
                    nc.scalar.activation(
                        out=e_rows[0:1, b, tok0:tok0 + 512],
                        in_=score_ps[:, :], func=AF.Exp,
                        accum_out=dparts[0:1, b, blk:blk + 1],
                    )

                    # ---- e columns via PE transpose (tiny) ----
                    eT_ps = peTp.tile([128, TPB], f32)
                    for c in range(TPB):
                        nc.tensor.matmul(
                            eT_ps[:, c:c + 1],
                            e_rows[0:1, b, tok0 + 128 * c:tok0 + 128 * (c + 1)],
                            ident[0:1, 0:1],
                            is_transpose=True,
                            start=(c == 0), stop=(c == TPB - 1),
                        )
                    eT = eTsp.tile([128, TPB], f16)
                    nc.vector.tensor_copy(out=eT, in_=eT_ps[:, :])

                    # ---- mm3: ctx += e.T @ keys (natural layout) ----
                    for tt in range(TPB):
                        for half in range(2):
                            nc.tensor.matmul(
                                ctx_ps[:, 512 * half:512 * (half + 1)],
                                eT[:, tt:tt + 1],
                                ktiles[tt][:, 512 * half:512 * (half + 1)],
                                start=(blk == 0 and tt == 0),
                                stop=(blk == NBLK - 1 and tt == TPB - 1),
                            )

                # ---- batch epilogue: denom -> rinv -> scale ctx ----
                nc.vector.tensor_reduce(
                    out=denom[0:1, b:b + 1], in_=dparts[0:1, b, :],
                    axis=mybir.AxisListType.X, op=mybir.AluOpType.add,
                )
                nc.vector.reciprocal(out=rinv[0:1, b:b + 1], in_=denom[0:1, b:b + 1])
                nc.vector.tensor_scalar_mul(
                    out=ctx_sb[0:1, b, :], in0=ctx_ps[:, :],
                    scalar1=rinv[0:1, b:b + 1],
                )

            # ---- weights output ----
            for b in range(BPC):
                nc.vector.tensor_scalar_mul(
                    out=w_rows[0:1, b, :], in0=e_rows[0:1, b, :],
                    scalar1=rinv[0:1, b:b + 1],
                )
            nc.gpsimd.dma_start(out=wout_d[:, :, :], in_=w_rows[:, :, :])
            nc.gpsimd.dma_start(out=ctx_d[:, :, :], in_=ctx_sb[:, :, :])
            nc.gpsimd.dma_start(out=er_d[:, :, :], in_=e_rows[:, :, :])
            nc.gpsimd.dma_start(out=dp_d[:, :, :], in_=dparts[:, :, :])

    nc.compile()
    return nc


def _get_nc():
    if "nc" not in _CACHE:
        _CACHE["nc"] = _build()
    return _CACHE["nc"]


def _in_maps(query, keys, Wq_w, Wq_b, Wk_w, Wk_b, Va_w, Va_b):
    q_proj = query[:, 0, :] @ Wq_w.T + Wq_b          # [B, H]
    qkb = q_proj + Wk_b                              # [B, H]
    wkT = np.ascontiguousarray(Wk_w.T).reshape(8, 128, H).astype(np.float16)
    vaT = Va_w.reshape(4, 128)
    ident = np.ones((1, 1), dtype=np.float32)
    keys16 = keys.astype(np.float16)

    in_maps = []
    for c in range(NCORES):
        bs = slice(c * BPC, (c + 1) * BPC)
        qkbT = np.ascontiguousarray(qkb[bs].T.reshape(4, 128, BPC))
        in_maps.append({
            "keys16": keys16[bs],
            "wkT": wkT,
            "qkbT": qkbT,
            "vaT": vaT,
            "ident": ident,
        })
    return in_maps


def kernel(query, keys, Wq_w, Wq_b, Wk_w, Wk_b, Va_w, Va_b):
    from concourse.bass_utils import run_bass_kernel_spmd

    query = np.asarray(query, dtype=np.float32)
    keys = np.asarray(keys, dtype=np.float32)
    Wq_w = np.asarray(Wq_w, dtype=np.float32)
    Wq_b = np.asarray(Wq_b, dtype=np.float32)
    Wk_w = np.asarray(Wk_w, dtype=np.float32)
    Wk_b = np.asarray(Wk_b, dtype=np.float32)
    Va_w = np.asarray(Va_w, dtype=np.float32)
    Va_b = np.asarray(Va_b, dtype=np.float32)

    in_maps = _in_maps(query, keys, Wq_w, Wq_b, Wk_w, Wk_b, Va_w, Va_b)
    nc = _get_nc()
    res = run_bass_kernel_spmd(nc, in_maps, core_ids=list(range(NCORES)))

    context = np.zeros((B, 1, D2), np.float32)
    weights = np.zeros((B, 1, S), np.float32)
    for c in range(NCORES):
        bs = slice(c * BPC, (c + 1) * BPC)
        context[bs, 0, :] = res.results[c]["ctx"][0]
        weights[bs, 0, :] = res.results[c]["wout"][0]
    return (context, weights)


# revision 10
# speedup vs baseline: 1.4928x; 1.4928x over previous
"""Bahdanau attention TRN2 kernel (v2: fp16 keys + DMA-transpose).

Full inputs -> full outputs. Data-parallel over batch B=32 across 8 cores
(4 batches/core). Weights replicated (tiny, pre-transposed/cast on host).

Per core, per 512-token block (4 blocks/batch):
  DMA  keys tiles [128,1024] fp16 (natural layout, for context matmul)
  DMA  keysT [128 d, 512 tok] fp16 via x-bar DMA transpose (for k_proj)
  PE   mm1: k_projT[h, tok] = sum_d WkT[d,h].T @ keysT[d,tok]   (fp16 -> fp32 psum)
  ACT  tanh(k_proj + q_proj + Wk_b) with per-partition bias column
  PE   mm2: scores[1, tok] = sum_h VaT[h,1].T @ tanh[h,tok]      (fp32r)
  ACT  exp (no max subtract; scores bounded ~|4| for N(0,1) inputs)
       with fused accum_out partial denominator
  PE   transpose e-row chunks -> e columns [tok,1] (tiny)
  PE   mm3: ctx[1, d] += e[tok,1].T @ keys[tok,d]                (fp16)
Softmax bookkeeping lives entirely on partition 0 (no cross-partition ops).
weights = e * (1/denom); ctx scaled by 1/denom per batch.
"""

import numpy as np

B, S, H = 32, 2048, 512
D2 = 2 * H          # 1024
NCORES = 8
BPC = B // NCORES   # 4 batches per core
NBLK = 4            # 512-token blocks per batch
TPB = 4             # 128-token tiles per block

_CACHE = {}


def _build():
    import concourse.bacc as bacc
    import concourse.tile as tile
    from concourse import mybir

    f16 = mybir.dt.float16
    f32 = mybir.dt.float32
    f32r = mybir.dt.float32r
    AF = mybir.ActivationFunctionType

    nc = bacc.Bacc(None, target_bir_lowering=False)

    keys_d = nc.dram_tensor("keys16", [BPC, S, D2], f16, kind="ExternalInput")
    wkT_d = nc.dram_tensor("wkT", [8, 128, H], f16, kind="ExternalInput")
    qkbT_d = nc.dram_tensor("qkbT", [4, 128, BPC], f32, kind="ExternalInput")
    vaT_d = nc.dram_tensor("vaT", [4, 128], f32r, kind="ExternalInput")
    ident_d = nc.dram_tensor("ident", [1, 1], f32, kind="ExternalInput")
    ctx_d = nc.dram_tensor("ctx", [1, BPC, D2], f32, kind="ExternalOutput")
    wout_d = nc.dram_tensor("wout", [1, BPC, S], f32, kind="ExternalOutput")
    er_d = nc.dram_tensor("er_dbg", [1, BPC, S], f32, kind="ExternalOutput")
    dp_d = nc.dram_tensor("dp_dbg", [1, BPC, NBLK], f32, kind="ExternalOutput")

    keys_r = keys_d.rearrange("b (t p) d -> b t p d", p=128)  # t = 16 token tiles

    with tile.TileContext(nc) as tc:
        with (
            tc.tile_pool(name="const", bufs=1) as constp,
            tc.tile_pool(name="keys", bufs=8) as keysp,
            tc.tile_pool(name="keysT", bufs=2) as keysTp,
            tc.tile_pool(name="tt", bufs=2) as ttp,
            tc.tile_pool(name="small", bufs=1) as smallp,
            tc.tile_pool(name="eTs", bufs=2) as eTsp,
            tc.tile_pool(name="pkp", bufs=3, space="PSUM") as pkpp,
            tc.tile_pool(name="psc", bufs=2, space="PSUM") as pscp,
            tc.tile_pool(name="peT", bufs=1, space="PSUM") as peTp,
            tc.tile_pool(name="pctx", bufs=1, space="PSUM") as pctxp,
        ):
            # ---- constants ----
            wkT = constp.tile([128, 8, H], f16)       # [d_in_chunk, di, h]
            qkbT = constp.tile([128, 4, BPC], f32)    # [h_in_chunk, hi, b]
            vaT = constp.tile([128, 4], f32r)         # [h_in_chunk, hi]
            ident = constp.tile([1, 1], f32)
            nc.gpsimd.dma_start(out=wkT, in_=wkT_d.rearrange("k p h -> p k h"))
            nc.gpsimd.dma_start(out=qkbT, in_=qkbT_d.rearrange("k p b -> p k b"))
            nc.gpsimd.dma_start(out=vaT, in_=vaT_d.rearrange("k p -> p k"))
            nc.gpsimd.dma_start(out=ident, in_=ident_d[:, :])

            # softmax bookkeeping, all on partition 0
            e_rows = smallp.tile([1, BPC, S], f32)
            w_rows = smallp.tile([1, BPC, S], f32)
            dparts = smallp.tile([1, BPC, NBLK], f32)
            denom = smallp.tile([1, BPC], f32)
            rinv = smallp.tile([1, BPC], f32)
            ctx_sb = smallp.tile([1, BPC, D2], f32)

            for b in range(BPC):
                ctx_ps = pctxp.tile([1, D2], f32)
                keysTb = keysTp.tile([128, 8, S], f16)
                for di in range(8):
                    nc.sync.dma_start_transpose(
                        out=keysTb[:, di, :],
                        in_=keys_d[b, :, 128 * di:128 * (di + 1)],
                    )
                for blk in range(NBLK):
                    tok0 = 512 * blk
                    # ---- load 4 natural keys tiles ----
                    ktiles = []
                    for tt in range(TPB):
                        kt = keysp.tile([128, D2], f16, tag="ktile")
                        nc.gpsimd.dma_start(out=kt, in_=keys_r[b, blk * TPB + tt])
                        ktiles.append(kt)

                    # ---- mm1 + tanh + mm2 ----
                    score_ps = pscp.tile([1, 512], f32)
                    for hi in range(4):
                        kp = pkpp.tile([128, 512], f32)
                        for di in range(8):
                            nc.tensor.matmul(
                                kp[:, :],
                                wkT[:, di, 128 * hi:128 * (hi + 1)],
                                keysTb[:, di, tok0:tok0 + 512],
                                start=(di == 0), stop=(di == 7),
                            )
                        th = ttp.tile([128, 512], f32r)
                        nc.scalar.activation(
                            out=th, in_=kp[:, :], func=AF.Tanh,
                            bias=qkbT[:, hi, b:b + 1],
                        )
                        nc.tensor.matmul(
                            score_ps[:, :],
                            vaT[:, hi:hi + 1],
                            th[:, :],
                            start=(hi == 0), stop=(hi == 3),
                        )

                    # ---- exp + partial denom (fused accumulate) ----
                    nc.scalar.activation(
                        out=e_rows[0:1, b, tok0:tok0 + 512],
                        in_=score_ps[:, :], func=AF.Exp,
                        accum_out=dparts[0:1, b, blk:blk + 1],
                    )

                    # ---- e columns via PE transpose (tiny) ----
                    eT_ps = peTp.tile([128, TPB], f32)
                    for c in range(TPB):
                        nc.tensor.matmul(
                            eT_ps[:, c:c + 1],
                            e_rows[0:1, b, tok0 + 128 * c:tok0 + 128 * (c + 1)],
                            ident[0:1, 0:1],
                            is_transpose=True,
                            start=(c == 0), stop=(c == TPB - 1),
                        )
                    eT = eTsp.tile([128, TPB], f16)
                    nc.vector.tensor_copy(out=eT, in_=eT_ps[:, :])

                    # ---- mm3: ctx += e.T @ keys (natural layout) ----
                    for tt in range(TPB):
                        for half in range(2):
                            nc.tensor.matmul(
                                ctx_ps[:, 512 * half:512 * (half + 1)],
                                eT[:, tt:tt + 1],
                                ktiles[tt][:, 512 * half:512 * (half + 1)],
                                start=(blk == 0 and tt == 0),
                                stop=(blk == NBLK - 1 and tt == TPB - 1),
                            )

                # ---- batch epilogue: denom -> rinv -> scale ctx ----
                nc.vector.tensor_reduce(
                    out=denom[0:1, b:b + 1], in_=dparts[0:1, b, :],
                    axis=mybir.AxisListType.X, op=mybir.AluOpType.add,
                )
                nc.vector.reciprocal(out=rinv[0:1, b:b + 1], in_=denom[0:1, b:b + 1])
                nc.vector.tensor_scalar_mul(
                    out=ctx_sb[0:1, b, :], in0=ctx_ps[:, :],
                    scalar1=rinv[0:1, b:b + 1],
                )

            # ---- weights output ----
            for b in range(BPC):
                nc.vector.tensor_scalar_mul(
                    out=w_rows[0:1, b, :], in0=e_rows[0:1, b, :],
                    scalar1=rinv[0:1, b:b + 1],
                )
            nc.gpsimd.dma_start(out=wout_d[:, :, :], in_=w_rows[:, :, :])
            nc.gpsimd.dma_start(out=ctx_d[:, :, :], in_=ctx_sb[:, :, :])
            nc.gpsimd.dma_start(out=er_d[:, :, :], in_=e_rows[:, :, :])
            nc.gpsimd.dma_start(out=dp_d[:, :, :], in_=dparts[:, :, :])

    nc.compile()
    return nc


def _get_nc():
    if "nc" not in _CACHE:
        _CACHE["nc"] = _build()
    return _CACHE["nc"]


def _in_maps(query, keys, Wq_w, Wq_b, Wk_w, Wk_b, Va_w, Va_b):
    q_proj = query[:, 0, :] @ Wq_w.T + Wq_b          # [B, H]
    qkb = q_proj + Wk_b                              # [B, H]
    wkT = np.ascontiguousarray(Wk_w.T).reshape(8, 128, H).astype(np.float16)
    vaT = Va_w.reshape(4, 128)
    ident = np.ones((1, 1), dtype=np.float32)
    keys16 = keys.astype(np.float16)

    in_maps = []
    for c in range(NCORES):
        bs = slice(c * BPC, (c + 1) * BPC)
        qkbT = np.ascontiguousarray(qkb[bs].T.reshape(4, 128, BPC))
        in_maps.append({
            "keys16": keys16[bs],
            "wkT": wkT,
            "qkbT": qkbT,
            "vaT": vaT,
            "ident": ident,
        })
    return in_maps


def kernel(query, keys, Wq_w, Wq_b, Wk_w, Wk_b, Va_w, Va_b):
    from concourse.bass_utils import run_bass_kernel_spmd

    query = np.asarray(query, dtype=np.float32)
    keys = np.asarray(keys, dtype=np.float32)
    Wq_w = np.asarray(Wq_w, dtype=np.float32)
    Wq_b = np.asarray(Wq_b, dtype=np.float32)
    Wk_w = np.asarray(Wk_w, dtype=np.float32)
    Wk_b = np.asarray(Wk_b, dtype=np.float32)
    Va_w = np.asarray(Va_w, dtype=np.float32)
    Va_b = np.asarray(Va_b, dtype=np.float32)

    in_maps = _in_maps(query, keys, Wq_w, Wq_b, Wk_w, Wk_b, Va_w, Va_b)
    nc = _get_nc()
    res = run_bass_kernel_spmd(nc, in_maps, core_ids=list(range(NCORES)))

    context = np.zeros((B, 1, D2), np.float32)
    weights = np.zeros((B, 1, S), np.float32)
    for c in range(NCORES):
        bs = slice(c * BPC, (c + 1) * BPC)
        context[bs, 0, :] = res.results[c]["ctx"][0]
        weights[bs, 0, :] = res.results[c]["wout"][0]
    return (context, weights)
